# revision 1
# baseline (speedup 1.0000x reference)
"""Causal self-attention (B=4, T=2048, C=1024, H=16, D=64) on 8 TRN2 NeuronCores.

Sharding: core = (batch b, head-group g) with b = core // 2, g = core % 2.
Each core computes heads [8g, 8g+8) of batch b and produces the partial
out-projection (C, T) for its head group; the host sums the two head-group
partials per batch and adds the output bias.

On-device layout notes:
- All activations/weights enter the PE as fp16; PSUM accumulates fp32.
- q/k are produced "transposed" (feature on partitions, t on free dim) so
  scores can be computed as ST[s, t] = k^T q with no transposes anywhere.
- RoPE feature permutation per head: rows [e0..e15, o0..o15, e16..e31,
  o16..o31] (e=even/cos-lane of pair i, o=odd). The pair swap is then a
  16-row swap inside each 32-partition quadrant -> one DVE stream_shuffle.
- Softmax runs unnormalized in the (s, t) orientation: E = exp(S/8); the
  per-t denominator is produced by an extra all-ones column appended to V
  (M=65 in the att@V matmul); normalization divides at the end.
- Causal masking: fully-masked (s, t) tiles are skipped; diagonal tiles are
  zeroed elementwise post-exp with gpsimd.affine_select.
"""

import numpy as np

B, T, C = 4, 2048, 1024
H, D = 16, 64
N_CORES = 8
HPG = H // 2            # heads per core (group)
NCHUNK = 4              # head-pair chunks per core
KT = 8                  # k-tiles of 128 over C
KT_AUG = 9              # + bias/ones k-tile
TT = 4                  # t-tiles of 512 over T
NT = 512                # t tile (matmul N)
VS = 66                 # v column stride per head (64 dims + ones + pad)
VW = HPG * VS           # 528 v columns per k-chunk block
ROPE_BASE = 10000.0

_CACHE = {}


def _d_of_r(r):
    # row r (0..63) inside a head's 64 rotated rows -> original head dim d
    f = (r // 32) * 16 + (r % 16)
    return 2 * f + (1 if (r % 32) >= 16 else 0)


def _f_of_p(p):
    # partition p (0..127) -> rope frequency index
    return ((p // 32) % 2) * 16 + (p % 16)


def _build_nc():
    import concourse.bass as bass  # noqa: F401
    import concourse.tile as tile
    from concourse import bacc, mybir
    from contextlib import ExitStack

    f16 = mybir.dt.float16
    f32 = mybir.dt.float32

    nc = bacc.Bacc(
        "TRN2",
        target_bir_lowering=False,
        debug=False,
        enable_asserts=True,
        num_devices=N_CORES,
    )

    xt_d = nc.dram_tensor("xt", (KT_AUG * 128, T), f16, kind="ExternalInput").ap()
    wqk_d = nc.dram_tensor("wqk", (128, KT * 1024), f16, kind="ExternalInput").ap()
    wv_d = nc.dram_tensor("wv", (128, KT_AUG * VW), f16, kind="ExternalInput").ap()
    wo_d = nc.dram_tensor("wo", (128, NCHUNK * 1024), f16, kind="ExternalInput").ap()
    bqk_d = nc.dram_tensor("bqk", (128, 16), f32, kind="ExternalInput").ap()
    cs_d = nc.dram_tensor("cs", (128, T), f16, kind="ExternalInput").ap()
    css_d = nc.dram_tensor("css", (128, T), f16, kind="ExternalInput").ap()
    ot_d = nc.dram_tensor("ot", (1024, T), f32, kind="ExternalOutput").ap()

    SHUF = list(range(16, 32)) + list(range(0, 16))

    with tile.TileContext(nc) as tc:
        with ExitStack() as ctx, nc.allow_low_precision("fp16 attention pipeline"):
            consts = ctx.enter_context(tc.tile_pool(name="consts", bufs=1))
            qk_pool = ctx.enter_context(tc.tile_pool(name="qk", bufs=2))
            rtmp = ctx.enter_context(tc.tile_pool(name="rtmp", bufs=4))
            e_pool = ctx.enter_context(tc.tile_pool(name="e", bufs=8))
            small = ctx.enter_context(tc.tile_pool(name="small", bufs=3))
            osb = ctx.enter_context(tc.tile_pool(name="osb", bufs=6))
            ps_big = ctx.enter_context(tc.tile_pool(name="psbig", bufs=2, space="PSUM"))
            ps_s = ctx.enter_context(tc.tile_pool(name="pss", bufs=2, space="PSUM"))
            ps_y = ctx.enter_context(tc.tile_pool(name="psy", bufs=1, space="PSUM"))

            # ---- resident tiles + input DMA ----
            xt = consts.tile([128, KT_AUG * T], f16)
            for kc in range(KT_AUG):
                for i in range(2):
                    nc.sync.dma_start(xt[:, kc * T + i * 1024: kc * T + (i + 1) * 1024],
                                      xt_d[kc * 128:(kc + 1) * 128, i * 1024:(i + 1) * 1024])
            def dma_split(dst, src, width, parts):
                step = width // parts
                for i in range(parts):
                    nc.sync.dma_start(dst[:, i * step:(i + 1) * step],
                                      src[:, i * step:(i + 1) * step])

            wqk = consts.tile([128, KT * 1024], f16)
            dma_split(wqk, wqk_d, KT * 1024, 8)
            wv = consts.tile([128, KT_AUG * VW], f16)
            dma_split(wv, wv_d, KT_AUG * VW, 4)
            wo = consts.tile([128, NCHUNK * 1024], f16)
            dma_split(wo, wo_d, NCHUNK * 1024, 4)
            bqk = consts.tile([128, 16], f32)
            nc.sync.dma_start(bqk[:], bqk_d[:])
            cs = consts.tile([128, T], f16)
            dma_split(cs, cs_d, T, 2)
            css = consts.tile([128, T], f16)
            dma_split(css, css_d, T, 2)
            v_sb = consts.tile([128, 16 * VW], f16)
            y_all = consts.tile([128, NCHUNK * T], f16)

            # ---- phase 0: V projection for all 8 heads ----
            with nc.named_scope("vproj"):
                for m in range(16):  # 128-row t-slices
                    psa = ps_big.tile([128, 512], f32, tag="big")
                    psb = ps_s.tile([128, 1024], f32, tag="s")
                    for kc in range(KT_AUG):
                        lhs = xt[:, kc * T + m * 128: kc * T + (m + 1) * 128]
                        nc.tensor.matmul(psa[:], lhs, wv[:, kc * VW: kc * VW + 512],
                                         start=(kc == 0), stop=(kc == KT_AUG - 1))
                        nc.tensor.matmul(psb[:, 0:16], lhs, wv[:, kc * VW + 512: (kc + 1) * VW],
                                         start=(kc == 0), stop=(kc == KT_AUG - 1))
                    nc.vector.tensor_copy(v_sb[:, m * VW: m * VW + 512], psa[:])
                    nc.vector.tensor_copy(v_sb[:, m * VW + 512: (m + 1) * VW], psb[:, 0:16])

            for c in range(NCHUNK):
                # ---- phase 1: q/k projection + RoPE for heads (2c, 2c+1) ----
                rq = qk_pool.tile([128, T], f16, tag="rq")
                rk = qk_pool.tile([128, T], f16, tag="rk")
                with nc.named_scope("qkrope"):
                    for tt in range(TT):
                        t0 = tt * NT
                        for which, dst in ((0, rq), (1, rk)):
                            ps = ps_big.tile([128, 512], f32, tag="big")
                            for kc in range(KT):
                                lhsT = wqk[:, kc * 1024 + c * 256 + which * 128: kc * 1024 + c * 256 + which * 128 + 128]
                                rhs = xt[:, kc * T + t0: kc * T + t0 + NT]
                                nc.tensor.matmul(ps[:], lhsT, rhs, start=(kc == 0), stop=(kc == KT - 1))
                            bcol = bqk[:, c * 4 + which * 2: c * 4 + which * 2 + 1]
                            bswp = bqk[:, c * 4 + which * 2 + 1: c * 4 + which * 2 + 2]
                            s_t = rtmp.tile([128, 512], f32, tag="s")
                            nc.vector.stream_shuffle(s_t[:], ps[:], SHUF)
                            x1 = rtmp.tile([128, 512], f16, tag="x1")
                            nc.vector.scalar_tensor_tensor(
                                out=x1[:], in0=ps[:], scalar=bcol, in1=cs[:, t0:t0 + NT],
                                op0=mybir.AluOpType.add, op1=mybir.AluOpType.mult)
                            x2 = rtmp.tile([128, 512], f16, tag="x2")
                            nc.vector.scalar_tensor_tensor(
                                out=x2[:], in0=s_t[:], scalar=bswp, in1=css[:, t0:t0 + NT],
                                op0=mybir.AluOpType.add, op1=mybir.AluOpType.mult)
                            nc.vector.tensor_add(dst[:, t0:t0 + NT], x1[:], x2[:])

                # ---- phase 2: attention for this chunk ----
                # Both heads of the chunk share 1024-wide paired tiles:
                # cols [0:512) = head 2c, [512:1024) = head 2c+1.
                with nc.named_scope("attn"):
                    for tt in range(TT):
                        t0 = tt * NT
                        sc_max = (t0 + NT) // 128
                        yp = ps_y.tile([65, 1024], f32, tag="y")
                        for sc in range(sc_max):
                            s0 = sc * 128
                            dlt = max(0, s0 - t0)  # first causal-valid col in tile
                            w = NT - dlt
                            sp = ps_s.tile([128, 1024], f32, tag="s")
                            nc.tensor.matmul(sp[:, dlt:NT], rk[0:64, s0:s0 + 128],
                                             rq[0:64, t0 + dlt:t0 + NT],
                                             start=True, stop=True, tile_position=(0, 0))
                            nc.tensor.matmul(sp[:, NT + dlt:2 * NT], rk[64:128, s0:s0 + 128],
                                             rq[64:128, t0 + dlt:t0 + NT],
                                             start=True, stop=True, tile_position=(64, 0))
                            e_t = e_pool.tile([128, 1024], f16)
                            s3 = sp[:].rearrange("p (a b) -> p a b", a=2)[:, :, dlt:]
                            e3 = e_t[:].rearrange("p (a b) -> p a b", a=2)[:, :, dlt:]
                            nc.scalar.activation(e3, s3, mybir.ActivationFunctionType.Exp,
                                                 bias=0.0, scale=0.125)
                            if s0 + 127 > t0:
                                # keep iff j' >= p  (j' is offset within the
                                # shrunken width; diagonal starts at col dlt)
                                nc.gpsimd.affine_select(
                                    out=e3, in_=e3,
                                    compare_op=mybir.AluOpType.is_ge,
                                    fill=0.0, base=0,
                                    pattern=[[0, 2], [1, w]], channel_multiplier=-1)
                            for h in range(2):
                                vcol = sc * VW + VS * (2 * c + h)
                                nc.tensor.matmul(yp[:, h * NT + dlt:(h + 1) * NT],
                                                 v_sb[:, vcol: vcol + 65],
                                                 e_t[:, h * NT + dlt:(h + 1) * NT],
                                                 start=(sc == 0), stop=(sc == sc_max - 1),
                                                 skip_group_check=True)
                        # single read of yp frees its PSUM slot immediately;
                        # normalization then runs SBUF-only (2x DVE mode)
                        yc = small.tile([65, 1024], f32, tag="yc")
                        nc.vector.tensor_copy(yc[:], yp[:])
                        rd = small.tile([1, 1024], f32, tag="rd")
                        nc.vector.reciprocal(rd[:], yc[64:65, :])
                        rbc = small.tile([64, 1024], f32, tag="rbc")
                        nc.gpsimd.partition_broadcast(rbc[:], rd[:])
                        for h in range(2):
                            nc.vector.tensor_mul(
                                y_all[h * 64:(h + 1) * 64, c * T + t0: c * T + t0 + NT],
                                yc[0:64, h * NT:(h + 1) * NT],
                                rbc[:, h * NT:(h + 1) * NT])

            # ---- phase 3: output projection (partial over this core's heads) ----
            with nc.named_scope("oproj"):
                for ct in range(8):
                    for tt in range(TT):
                        t0 = tt * NT
                        po = ps_big.tile([128, 512], f32, tag="big")
                        for c in range(NCHUNK):
                            nc.tensor.matmul(po[:], wo[:, c * 1024 + ct * 128: c * 1024 + ct * 128 + 128],
                                             y_all[:, c * T + t0: c * T + t0 + NT],
                                             start=(c == 0), stop=(c == NCHUNK - 1))
                        ob = osb.tile([128, 512], f32)
                        nc.vector.tensor_copy(ob[:], po[:])
                        nc.sync.dma_start(ot_d[ct * 128:(ct + 1) * 128, t0:t0 + NT], ob[:])

    nc.compile()
    return nc


def _prep_inputs(x, qkv_w, qkv_b):
    """Build the 8 per-core input maps (all host-side numpy)."""
    x = np.asarray(x, dtype=np.float32)
    qkv_w = np.asarray(qkv_w, dtype=np.float32)
    qkv_b = np.asarray(qkv_b, dtype=np.float32)

    # xt per batch: (KT_AUG*128, T) fp16 with row 1024 = ones, rest of aug block 0
    xts = []
    for b in range(B):
        xa = np.zeros((KT_AUG * 128, T), dtype=np.float16)
        xa[:C] = x[b].T.astype(np.float16)
        xa[C] = 1.0
        xts.append(xa)

    r = np.arange(64)
    d_r = 2 * ((r // 32) * 16 + (r % 16)) + ((r % 32) >= 16)  # row -> head dim
    p = np.arange(128)
    f_p = ((p // 32) % 2) * 16 + (p % 16)

    ins_g = []
    for g in range(2):
        # wqk: [p, kc*1024 + c*256 + which*128 + m]
        wqk = np.empty((128, KT * 1024), dtype=np.float16)
        bqk = np.empty((128, 16), dtype=np.float32)
        for c in range(NCHUNK):
            for which in range(2):  # 0=q, 1=k
                rows = np.concatenate([
                    which * C + (8 * g + 2 * c + hh) * 64 + d_r for hh in range(2)
                ])  # 128 feature rows
                blk = qkv_w[rows, :]          # (128 feat, 1024 k)
                for kc in range(KT):
                    wqk[:, kc * 1024 + c * 256 + which * 128:
                        kc * 1024 + c * 256 + which * 128 + 128] = \
                        blk[:, kc * 128:(kc + 1) * 128].T.astype(np.float16)
                bc = qkv_b[rows].astype(np.float32)
                bqk[:, c * 4 + which * 2] = bc
                bqk[:, c * 4 + which * 2 + 1] = bc[p ^ 16]
        # wv: [p, kc*VW + col], col = VS*h + j
        wva = np.zeros((KT_AUG * 128, VW), dtype=np.float32)
        for h in range(HPG):
            rows = 2 * C + (8 * g + h) * 64 + np.arange(64)
            wva[:C, VS * h: VS * h + 64] = qkv_w[rows, :].T
            wva[C, VS * h: VS * h + 64] = qkv_b[rows]
            wva[C, VS * h + 64] = 1.0
        wv = np.empty((128, KT_AUG * VW), dtype=np.float16)
        for kc in range(KT_AUG):
            wv[:, kc * VW:(kc + 1) * VW] = wva[kc * 128:(kc + 1) * 128].astype(np.float16)
        ins_g.append((wqk, bqk, wv))

    # rope tables
    inv_freq = (1.0 / (ROPE_BASE ** (np.arange(0, D, 2) / D))).astype(np.float64)
    t = np.arange(T, dtype=np.float64)
    ang = t[None, :] * inv_freq[f_p][:, None]          # (128, T)
    cs = np.cos(ang).astype(np.float16)
    sgn = np.where((p % 32) < 16, -1.0, 1.0)[:, None]
    css = (sgn * np.sin(ang)).astype(np.float16)

    return xts, ins_g, cs, css


def _prep_wo(out_w, g):
    out_w = np.asarray(out_w, dtype=np.float32)
    wo = np.empty((128, NCHUNK * 1024), dtype=np.float16)
    for c in range(NCHUNK):
        rows = np.concatenate([(8 * g + 2 * c + hh) * 64 + np.arange(64) for hh in range(2)])
        wo[:, c * 1024:(c + 1) * 1024] = out_w[:, rows].astype(np.float16).T
    return wo


def kernel(x, qkv_w, qkv_b, out_w, out_b):
    from concourse.bass_utils import run_bass_kernel_spmd

    if "nc" not in _CACHE:
        _CACHE["nc"] = _build_nc()
    nc = _CACHE["nc"]

    xts, ins_g, cs, css = _prep_inputs(x, qkv_w, qkv_b)
    wos = [_prep_wo(out_w, g) for g in range(2)]
    out_b = np.asarray(out_b, dtype=np.float32)

    in_maps = []
    for core in range(N_CORES):
        b, g = core // 2, core % 2
        wqk, bqk, wv = ins_g[g]
        in_maps.append({
            "xt": xts[b], "wqk": wqk, "wv": wv, "wo": wos[g],
            "bqk": bqk, "cs": cs, "css": css,
        })

    try:
        res = run_bass_kernel_spmd(nc, in_maps, core_ids=list(range(N_CORES)))
    except ModuleNotFoundError:
        # BASS_TRACE set but the NTFF profile hook isn't importable here
        import os
        os.environ["BASS_NEVER_TRACE"] = "1"
        res = run_bass_kernel_spmd(nc, in_maps, core_ids=list(range(N_CORES)))

    out = np.empty((B, T, C), dtype=np.float32)
    for b in range(B):
        pt = res.results[2 * b]["ot"] + res.results[2 * b + 1]["ot"]  # (C, T)
        out[b] = pt.T + out_b[None, :]
    return out



# revision 16
# speedup vs baseline: 1.1113x; 1.1113x over previous
"""Causal self-attention (B=4, T=2048, C=1024, H=16, D=64) on 8 TRN2 NeuronCores.

Sharding: core = (batch b, head-group g) with b = core // 2, g = core % 2.
Each core computes heads [8g, 8g+8) of batch b and produces the partial
out-projection (C, T) fp16 for its head group; the host sums the two
head-group partials per batch and adds the output bias.

Key speed tricks vs the fp16 baseline:
- All projections run as fp8e4 DoubleRow matmuls (2 k-tiles per pass at
  0.5 cycles/row). Accuracy is restored with a hi/lo split: operand =
  fp8(a) + fp8(a - fp8(a)); products keep the three O(1) terms
  (hi*hi + hi*lo + lo*hi), so inputs carry ~0.25% error instead of 3.6%.
- qkv bias is folded into an augmented k-tile pair (ones row in x8h
  ktiles 8/9 x bias_hi/bias_lo rows in the weights) so the rope stage is
  plain multiplies.
- att@V: exp() writes fp8 directly (activation cost is dtype-blind);
  V is stored as interleaved (v_hi, v_lo) fp8 pairs and a single
  DoubleRow matmul computes v_hi^T e + v_lo^T e per s-chunk using a
  stride-0 k-group broadcast of e.
- Scores stay fp16 (RoPE'd q/k would need a partition fold to reach the
  DoubleRow layout; not worth the DVE cost).
- Softmax is unnormalized with the denominator via an extra ones column
  of V; normalization happens once per (chunk, t-tile) in fp16.
"""

import numpy as np

B, T, C = 4, 2048, 1024
H, D = 16, 64
N_CORES = 8
HPG = H // 2            # heads per core (group)
NCHUNK = 4              # head-pair chunks per core
KT = 8                  # k-tiles of 128 over C
TT = 4                  # t-tiles of 512 over T
NT = 512                # t tile (matmul N)
VS = 66                 # v column stride per head (64 dims + ones + pad)
VW = HPG * VS           # 528 v columns per k-chunk block
ROPE_BASE = 10000.0

_CACHE = {}


def _build_nc():
    import concourse.bass as bass
    import concourse.tile as tile
    from concourse import bacc, mybir
    from contextlib import ExitStack

    f16 = mybir.dt.float16
    f32 = mybir.dt.float32
    f8 = mybir.dt.float8e4
    DR = mybir.MatmulPerfMode.DoubleRow

    nc = bacc.Bacc(
        "TRN2",
        target_bir_lowering=False,
        debug=False,
        enable_asserts=True,
        num_devices=N_CORES,
    )

    import os
    DEBUG_TAPS = bool(os.environ.get("KERNEL_DEBUG_TAPS"))
    if DEBUG_TAPS:
        dbg_rq_d = nc.dram_tensor("dbg_rq", (128, T), f16, kind="ExternalOutput").ap()
        dbg_v8_d = nc.dram_tensor("dbg_v8", (128, 16 * 2 * VW), f8, kind="ExternalOutput").ap()
        dbg_y_d = nc.dram_tensor("dbg_y", (128, NCHUNK * T), f16, kind="ExternalOutput").ap()
        dbg_e_d = nc.dram_tensor("dbg_e", (128, 1024), f8, kind="ExternalOutput").ap()

    x8h_d = nc.dram_tensor("x8h", (10 * 128, T), f8, kind="ExternalInput").ap()
    x8l_d = nc.dram_tensor("x8l", (KT * 128, T), f8, kind="ExternalInput").ap()
    wqk8h_d = nc.dram_tensor("wqk8h", (128, 10 * 1024), f8, kind="ExternalInput").ap()
    wqk8l_d = nc.dram_tensor("wqk8l", (128, KT * 1024), f8, kind="ExternalInput").ap()
    wv8h_d = nc.dram_tensor("wv8h", (128, 10 * VW), f8, kind="ExternalInput").ap()
    wv8l_d = nc.dram_tensor("wv8l", (128, KT * VW), f8, kind="ExternalInput").ap()
    wo_d = nc.dram_tensor("wo", (128, NCHUNK * 1024), f16, kind="ExternalInput").ap()
    cs_d = nc.dram_tensor("cs", (128, T), f16, kind="ExternalInput").ap()
    css_d = nc.dram_tensor("css", (128, T), f16, kind="ExternalInput").ap()
    ot_d = nc.dram_tensor("ot", (1024, T), f16, kind="ExternalOutput").ap()

    SHUF = list(range(16, 32)) + list(range(0, 16))

    def slot0(ap, w):
        # [K, 2, w] view of ap with a stride-0 k-group dim (broadcast)
        return bass.AP(ap.tensor, ap.offset, [list(ap.ap[0]), [0, 2], [1, w]])

    with tile.TileContext(nc) as tc:
        with ExitStack() as ctx, nc.allow_low_precision("fp8 attention pipeline"):
            consts = ctx.enter_context(tc.tile_pool(name="consts", bufs=1))
            qk_pool = ctx.enter_context(tc.tile_pool(name="qk", bufs=2))
            rtmp = ctx.enter_context(tc.tile_pool(name="rtmp", bufs=4))
            e_pool = ctx.enter_context(tc.tile_pool(name="e", bufs=8))
            small = ctx.enter_context(tc.tile_pool(name="small", bufs=3))
            osb = ctx.enter_context(tc.tile_pool(name="osb", bufs=6))
            ps_big = ctx.enter_context(tc.tile_pool(name="psbig", bufs=2, space="PSUM"))
            ps_s = ctx.enter_context(tc.tile_pool(name="pss", bufs=2, space="PSUM"))
            ps_y = ctx.enter_context(tc.tile_pool(name="psy", bufs=1, space="PSUM"))

            # ---- resident tiles + input DMA ----
            x8h = consts.tile([128, 10 * T], f8)
            for kc in range(10):
                nc.sync.dma_start(x8h[:, kc * T:(kc + 1) * T],
                                  x8h_d[kc * 128:(kc + 1) * 128, :])
            x8l = consts.tile([128, KT * T], f8)
            for kc in range(KT):
                nc.sync.dma_start(x8l[:, kc * T:(kc + 1) * T],
                                  x8l_d[kc * 128:(kc + 1) * 128, :])

            def dma_split(dst, src, width, parts):
                step = width // parts
                for i in range(parts):
                    nc.sync.dma_start(dst[:, i * step:(i + 1) * step],
                                      src[:, i * step:(i + 1) * step])

            wqk8h = consts.tile([128, 10 * 1024], f8)
            dma_split(wqk8h, wqk8h_d, 10 * 1024, 5)
            wqk8l = consts.tile([128, KT * 1024], f8)
            dma_split(wqk8l, wqk8l_d, KT * 1024, 4)
            wv8h = consts.tile([128, 10 * VW], f8)
            dma_split(wv8h, wv8h_d, 10 * VW, 2)
            wv8l = consts.tile([128, KT * VW], f8)
            dma_split(wv8l, wv8l_d, KT * VW, 2)
            wo = consts.tile([128, NCHUNK * 1024], f16)
            dma_split(wo, wo_d, NCHUNK * 1024, 4)
            cs = consts.tile([128, T], f16)
            dma_split(cs, cs_d, T, 2)
            css = consts.tile([128, T], f16)
            dma_split(css, css_d, T, 2)

            v8 = consts.tile([128, 16 * 2 * VW], f8)   # (m, hi|lo, col)
            y_all = consts.tile([128, NCHUNK * T], f16)

            x8h3 = x8h[:].rearrange("p (kc t) -> p kc t", kc=10)
            x8l3 = x8l[:].rearrange("p (kc t) -> p kc t", kc=KT)
            wv8h3 = wv8h[:].rearrange("p (kc c) -> p kc c", kc=10)
            wv8l3 = wv8l[:].rearrange("p (kc c) -> p kc c", kc=KT)
            wqk8h3 = wqk8h[:].rearrange("p (kc c) -> p kc c", kc=10)
            wqk8l3 = wqk8l[:].rearrange("p (kc c) -> p kc c", kc=KT)

            # ---- phase 0: V projection for all 8 heads (fp8 DR, 3 products + aug) ----
            with nc.named_scope("vproj"):
                for m in range(16):  # 128-row t-slices
                    psa = ps_big.tile([128, 512], f32, tag="big")
                    psb = ps_s.tile([128, 1024], f32, tag="s")
                    mm = []  # (lhsT_pair, rhs_pair_a, rhs_pair_b)
                    for j in range(KT // 2):
                        kk = slice(2 * j, 2 * j + 2)
                        mm.append((x8h3[:, kk, m * 128:(m + 1) * 128],
                                   wv8h3[:, kk, :]))          # hi*hi
                        mm.append((x8l3[:, kk, m * 128:(m + 1) * 128],
                                   wv8h3[:, kk, :]))          # lo_x*hi_w
                        mm.append((x8h3[:, kk, m * 128:(m + 1) * 128],
                                   wv8l3[:, kk, :]))          # hi_x*lo_w
                    mm.append((x8h3[:, 8:10, m * 128:(m + 1) * 128],
                               wv8h3[:, 8:10, :]))            # bias/ones aug
                    n = len(mm)
                    for i, (lhsT, rhs) in enumerate(mm):
                        nc.tensor.matmul(psa[:], lhsT, rhs[:, :, 0:512],
                                         start=(i == 0), stop=(i == n - 1),
                                         perf_mode=DR)
                        nc.tensor.matmul(psb[:, 0:16], lhsT, rhs[:, :, 512:VW],
                                         start=(i == 0), stop=(i == n - 1),
                                         perf_mode=DR)
                    # hi copy + lo residual (v8 layout: m*2*VW + {0:hi,VW:lo} + col)
                    base = m * 2 * VW
                    nc.vector.tensor_copy(v8[:, base: base + 512], psa[:])
                    nc.vector.tensor_copy(v8[:, base + 512: base + VW], psb[:, 0:16])
                    nc.vector.tensor_tensor(
                        out=v8[:, base + VW: base + VW + 512], in0=psa[:],
                        in1=v8[:, base: base + 512], op=mybir.AluOpType.subtract)
                    nc.vector.tensor_tensor(
                        out=v8[:, base + VW + 512: base + 2 * VW], in0=psb[:, 0:16],
                        in1=v8[:, base + 512: base + VW], op=mybir.AluOpType.subtract)

            v84 = v8[:].rearrange("p (m s c) -> p m s c", m=16, s=2)
            if DEBUG_TAPS:
                for i in range(4):
                    st = i * 8 * VW
                    nc.sync.dma_start(dbg_v8_d[:, st:st + 8 * VW], v8[:, st:st + 8 * VW])

            for c in range(NCHUNK):
                # ---- phase 1: q/k projection + RoPE for heads (2c, 2c+1) ----
                rq = qk_pool.tile([128, T], f16, tag="rq")
                rk = qk_pool.tile([128, T], f16, tag="rk")
                with nc.named_scope("qkrope"):
                    for tt in range(TT):
                        t0 = tt * NT
                        for which, dst in ((0, rq), (1, rk)):
                            cm = c * 256 + which * 128
                            ps = ps_big.tile([128, 512], f32, tag="big")
                            mm = []
                            for j in range(KT // 2):
                                kk = slice(2 * j, 2 * j + 2)
                                mm.append((wqk8h3[:, kk, cm:cm + 128],
                                           x8h3[:, kk, t0:t0 + NT]))
                                mm.append((wqk8h3[:, kk, cm:cm + 128],
                                           x8l3[:, kk, t0:t0 + NT]))
                                mm.append((wqk8l3[:, kk, cm:cm + 128],
                                           x8h3[:, kk, t0:t0 + NT]))
                            mm.append((wqk8h3[:, 8:10, cm:cm + 128],
                                       x8h3[:, 8:10, t0:t0 + NT]))  # bias aug
                            n = len(mm)
                            for i, (lhsT, rhs) in enumerate(mm):
                                nc.tensor.matmul(ps[:], lhsT, rhs,
                                                 start=(i == 0), stop=(i == n - 1),
                                                 perf_mode=DR)
                            s_t = rtmp.tile([128, 512], f32, tag="s")
                            nc.vector.stream_shuffle(s_t[:], ps[:], SHUF)
                            x1 = rtmp.tile([128, 512], f16, tag="x1")
                            nc.vector.tensor_mul(x1[:], ps[:], cs[:, t0:t0 + NT])
                            x2 = rtmp.tile([128, 512], f16, tag="x2")
                            nc.vector.tensor_mul(x2[:], s_t[:], css[:, t0:t0 + NT])
                            nc.vector.tensor_add(dst[:, t0:t0 + NT], x1[:], x2[:])

                if DEBUG_TAPS and c == 0:
                    nc.sync.dma_start(dbg_rq_d[:, :], rq[:, :])

                # ---- phase 2: attention for this chunk ----
                with nc.named_scope("attn"):
                    for tt in range(TT):
                        t0 = tt * NT
                        sc_max = (t0 + NT) // 128
                        yp = ps_y.tile([65, 1024], f32, tag="y")
                        for sc in range(sc_max):
                            s0 = sc * 128
                            dlt = max(0, s0 - t0)  # first causal-valid col in tile
                            w = NT - dlt
                            sp = ps_s.tile([128, 1024], f32, tag="s")
                            nc.tensor.matmul(sp[:, dlt:NT], rk[0:64, s0:s0 + 128],
                                             rq[0:64, t0 + dlt:t0 + NT],
                                             start=True, stop=True, tile_position=(0, 0))
                            nc.tensor.matmul(sp[:, NT + dlt:2 * NT], rk[64:128, s0:s0 + 128],
                                             rq[64:128, t0 + dlt:t0 + NT],
                                             start=True, stop=True, tile_position=(64, 0))
                            e_t = e_pool.tile([128, 1024], f8)
                            s3 = sp[:].rearrange("p (a b) -> p a b", a=2)[:, :, dlt:]
                            e3 = e_t[:].rearrange("p (a b) -> p a b", a=2)[:, :, dlt:]
                            nc.scalar.activation(e3, s3, mybir.ActivationFunctionType.Exp,
                                                 bias=0.0, scale=0.125)
                            if s0 + 127 > t0:
                                # keep iff j' >= p  (diagonal starts at col dlt)
                                nc.gpsimd.affine_select(
                                    out=e3, in_=e3,
                                    compare_op=mybir.AluOpType.is_ge,
                                    fill=0.0, base=0,
                                    pattern=[[0, 2], [1, w]], channel_multiplier=-1)
                            if DEBUG_TAPS and c == 0 and tt == 0 and sc == 0:
                                nc.sync.dma_start(dbg_e_d[:, :], e_t[:, :])
                            for h in range(2):
                                vc = VS * (2 * c + h)
                                nc.tensor.matmul(
                                    yp[:, h * NT + dlt:(h + 1) * NT],
                                    v84[:, sc, :, vc: vc + 65],
                                    slot0(e_t[:, h * NT + dlt: (h + 1) * NT], w),
                                    start=(sc == 0), stop=(sc == sc_max - 1),
                                    perf_mode=DR, skip_group_check=True)
                        # normalization in fp16 (2x DVE mode for the muls);
                        # denom stays f32 (PSUM) to dodge fp16 overflow at SV=8
                        yc = small.tile([64, 1024], f16, tag="yc")
                        nc.vector.tensor_copy(yc[:], yp[0:64, :])
                        rd = small.tile([1, 1024], f16, tag="rd")
                        nc.vector.reciprocal(rd[:], yp[64:65, :])
                        rbc = small.tile([64, 1024], f16, tag="rbc")
                        nc.gpsimd.partition_broadcast(rbc[:], rd[:])
                        for h in range(2):
                            nc.vector.tensor_mul(
                                y_all[h * 64:(h + 1) * 64, c * T + t0: c * T + t0 + NT],
                                yc[:, h * NT:(h + 1) * NT],
                                rbc[:, h * NT:(h + 1) * NT])

            if DEBUG_TAPS:
                for i in range(4):
                    st = i * T
                    nc.sync.dma_start(dbg_y_d[:, st:st + T], y_all[:, st:st + T])

            # ---- phase 3: output projection (partial over this core's heads) ----
            with nc.named_scope("oproj"):
                for ct in range(8):
                    for tt in range(TT):
                        t0 = tt * NT
                        po = ps_big.tile([128, 512], f32, tag="big")
                        for c in range(NCHUNK):
                            nc.tensor.matmul(po[:], wo[:, c * 1024 + ct * 128: c * 1024 + ct * 128 + 128],
                                             y_all[:, c * T + t0: c * T + t0 + NT],
                                             start=(c == 0), stop=(c == NCHUNK - 1))
                        ob = osb.tile([128, 512], f16)
                        nc.vector.tensor_copy(ob[:], po[:])
                        nc.sync.dma_start(ot_d[ct * 128:(ct + 1) * 128, t0:t0 + NT], ob[:])

    nc.compile()
    return nc


SQ = 32.0   # qk weight prescale (undone via cs/css tables)
SV = 8.0    # v weight prescale (cancels in softmax normalization)


def _fp8_split(a, np8, scale=1.0):
    """scale*a -> (hi, lo) fp8 with hi + lo ~= scale*a to ~0.13%.

    The scale lifts 0.02-magnitude weights above fp8e4's 2^-9 subnormal
    floor so the lo residual can actually represent the hi rounding error.
    """
    a = np.asarray(a, dtype=np.float32) * scale
    hi = a.astype(np8)
    lo = (a - hi.astype(np.float32)).astype(np8)
    return hi, lo


def _prep_inputs(x, qkv_w, qkv_b):
    """Build the per-core input maps (all host-side numpy)."""
    from concourse import mybir
    np8 = mybir.dt.np(mybir.dt.float8e4)

    x = np.asarray(x, dtype=np.float32)
    qkv_w = np.asarray(qkv_w, dtype=np.float32)
    qkv_b = np.asarray(qkv_b, dtype=np.float32)

    # x8 per batch: hi [10*128, T] (ktiles 8/9 = ones row), lo [8*128, T]
    x8hs, x8ls = [], []
    for b in range(B):
        xh = np.zeros((10 * 128, T), dtype=np8)
        xl = np.zeros((KT * 128, T), dtype=np8)
        hi, lo = _fp8_split(x[b].T, np8)
        xh[:C] = hi
        xl[:C] = lo
        xh[C] = np8(1.0)        # aug ktile 8: ones row
        xh[9 * 128] = np8(1.0)  # aug ktile 9: duplicate ones row
        x8hs.append(xh)
        x8ls.append(xl)

    r = np.arange(64)
    d_r = 2 * ((r // 32) * 16 + (r % 16)) + ((r % 32) >= 16)  # row -> head dim
    p = np.arange(128)
    f_p = ((p // 32) % 2) * 16 + (p % 16)

    ins_g = []
    for g in range(2):
        # wqk8: [p, kc*1024 + c*256 + which*128 + m]; kc8/9 = bias hi/lo on row 0
        wqkh = np.zeros((128, 10 * 1024), dtype=np8)
        wqkl = np.zeros((128, KT * 1024), dtype=np8)
        for c in range(NCHUNK):
            for which in range(2):  # 0=q, 1=k
                rows = np.concatenate([
                    which * C + (8 * g + 2 * c + hh) * 64 + d_r for hh in range(2)
                ])  # 128 feature rows
                blk = qkv_w[rows, :]          # (128 feat, 1024 k)
                cm = c * 256 + which * 128
                for kc in range(KT):
                    hi, lo = _fp8_split(blk[:, kc * 128:(kc + 1) * 128].T, np8, SQ)
                    wqkh[:, kc * 1024 + cm: kc * 1024 + cm + 128] = hi
                    wqkl[:, kc * 1024 + cm: kc * 1024 + cm + 128] = lo
                bh, bl = _fp8_split(qkv_b[rows], np8, SQ)
                wqkh[0, 8 * 1024 + cm: 8 * 1024 + cm + 128] = bh
                wqkh[0, 9 * 1024 + cm: 9 * 1024 + cm + 128] = bl
        # wv8: [p, kc*VW + VS*h + j]; kc8 = aug hi (bias+ones), kc9 = aug lo (bias)
        wva = np.zeros((KT * 128, VW), dtype=np.float32)
        aug = np.zeros((128, VW), dtype=np.float32)
        for h in range(HPG):
            rows = 2 * C + (8 * g + h) * 64 + np.arange(64)
            wva[:, VS * h: VS * h + 64] = qkv_w[rows, :].T
            aug[0, VS * h: VS * h + 64] = qkv_b[rows]
            aug[0, VS * h + 64] = 1.0
        wvh = np.zeros((128, 10 * VW), dtype=np8)
        wvl = np.zeros((128, KT * VW), dtype=np8)
        for kc in range(KT):
            hi, lo = _fp8_split(wva[kc * 128:(kc + 1) * 128], np8, SV)
            wvh[:, kc * VW:(kc + 1) * VW] = hi
            wvl[:, kc * VW:(kc + 1) * VW] = lo
        augh, augl = _fp8_split(aug, np8, SV)  # ones col becomes SV (exact in fp8)
        augl[0, VS * np.arange(HPG) + 64] = np8(0.0)  # ones col only in hi
        wvh[:, 8 * VW: 9 * VW] = augh
        wvh[:, 9 * VW: 10 * VW] = augl
        ins_g.append((wqkh, wqkl, wvh, wvl))

    # rope tables
    inv_freq = (1.0 / (ROPE_BASE ** (np.arange(0, D, 2) / D))).astype(np.float64)
    t = np.arange(T, dtype=np.float64)
    ang = t[None, :] * inv_freq[f_p][:, None]          # (128, T)
    cs = (np.cos(ang) / SQ).astype(np.float16)         # undo qk weight prescale
    sgn = np.where((p % 32) < 16, -1.0, 1.0)[:, None]
    css = (sgn * np.sin(ang) / SQ).astype(np.float16)

    return x8hs, x8ls, ins_g, cs, css


def _prep_wo(out_w, g):
    out_w = np.asarray(out_w, dtype=np.float32)
    wo = np.empty((128, NCHUNK * 1024), dtype=np.float16)
    for c in range(NCHUNK):
        rows = np.concatenate([(8 * g + 2 * c + hh) * 64 + np.arange(64) for hh in range(2)])
        wo[:, c * 1024:(c + 1) * 1024] = out_w[:, rows].astype(np.float16).T
    return wo


def _build_in_maps(x, qkv_w, qkv_b, out_w):
    x8hs, x8ls, ins_g, cs, css = _prep_inputs(x, qkv_w, qkv_b)
    wos = [_prep_wo(out_w, g) for g in range(2)]
    in_maps = []
    for core in range(N_CORES):
        b, g = core // 2, core % 2
        wqkh, wqkl, wvh, wvl = ins_g[g]
        in_maps.append({
            "x8h": x8hs[b], "x8l": x8ls[b],
            "wqk8h": wqkh, "wqk8l": wqkl,
            "wv8h": wvh, "wv8l": wvl,
            "wo": wos[g], "cs": cs, "css": css,
        })
    return in_maps


def kernel(x, qkv_w, qkv_b, out_w, out_b):
    from concourse.bass_utils import run_bass_kernel_spmd

    if "nc" not in _CACHE:
        _CACHE["nc"] = _build_nc()
    nc = _CACHE["nc"]

    in_maps = _build_in_maps(x, qkv_w, qkv_b, out_w)
    out_b = np.asarray(out_b, dtype=np.float32)

    try:
        res = run_bass_kernel_spmd(nc, in_maps, core_ids=list(range(N_CORES)))
    except ModuleNotFoundError:
        # BASS_TRACE set but the NTFF profile hook isn't importable here
        import os
        os.environ["BASS_NEVER_TRACE"] = "1"
        res = run_bass_kernel_spmd(nc, in_maps, core_ids=list(range(N_CORES)))

    out = np.empty((B, T, C), dtype=np.float32)
    for b in range(B):
        pt = (res.results[2 * b]["ot"].astype(np.float32)
              + res.results[2 * b + 1]["ot"].astype(np.float32))  # (C, T)
        out[b] = pt.T + out_b[None, :]
    return out
